# revision 1
# baseline (speedup 1.0000x reference)
"""Trainium2 Bass kernel for nn_Attention_86672440033867 (relative-position attention).

Sharding: head-parallel over 8 NeuronCores (1 head per core, all 16 batches).
Each core computes, for its head h:
  qkvT = w_qkv_h^T @ x^T           (M=96 chains -> qT/kT/vT rows)
  S^T  = k_b q_b^T                 (K=32 matmuls)
  P^T  = exp(SCALE*S^T) * exp(B)^T (ACT exp + DVE/GPSIMD multiply; bias via
                                    host-gathered exp(bias) table, batch-invariant)
  O^T  = v_b^T P^T (+ ones col -> softmax denominators)
  out_partial = (O^T / denom)^T @ w_out_h
Host sums the 8 partial projections and adds b_out.

The relative_index gather is resolved on the host: bias = table[relative_index]
is batch-independent, so exp(bias^T) is computed once per head and kept
resident in SBUF (2 MB bf16), amortized over all 16 batches.
"""
import numpy as np
import ml_dtypes
from contextlib import ExitStack, nullcontext

import concourse.bass as bass
import concourse.mybir as mybir
import concourse.tile as tile
from concourse import bacc
from concourse.bass_utils import run_bass_kernel_spmd

BF16 = mybir.dt.bfloat16
F32 = mybir.dt.float32

HEADS = 8
D = 32          # head dim
INP = 384
OUP = 384
SCALE = D ** -0.5
AF = mybir.ActivationFunctionType


def build_kernel(NB=16, N=1024, num_devices=8, loop_k=0):
    """Build the per-core Bass module. NB = total batches, N = tokens/batch."""
    assert NB % 4 == 0 and N % 128 == 0
    NJC = N // 128          # key chunks (128) per batch
    IH = min(512, N)        # query-column tile width
    NIH = N // IH           # query tiles per batch
    NTC = IH // 128         # token chunks (128) per query tile
    JG = min(2, NJC)        # j-chunks per exp/psum group
    NJG = (NJC + JG - 1) // JG
    TOK = NB * N

    nc = bacc.Bacc("TRN2", target_bir_lowering=False, num_devices=num_devices)

    xt_d = nc.dram_tensor("xt", [INP, TOK], BF16, kind="ExternalInput")
    wqkv_d = nc.dram_tensor("wqkv", [3, 128, 96], BF16, kind="ExternalInput")
    wout4_d = nc.dram_tensor("wout4", [128, OUP], BF16, kind="ExternalInput")
    expb_d = nc.dram_tensor("expb", [128, NJC, N], BF16, kind="ExternalInput")
    ident_d = nc.dram_tensor("ident", [128, 32], BF16, kind="ExternalInput")
    outp_d = nc.dram_tensor("outp", [TOK, OUP], BF16, kind="ExternalOutput")

    with tile.TileContext(nc) as tc, ExitStack() as ctx:
        const = ctx.enter_context(tc.tile_pool(name="const", bufs=1))
        big = ctx.enter_context(tc.tile_pool(name="big", bufs=1))

        wqkv_sb = const.tile([128, 3, 96], BF16)
        wout_sb = const.tile([128, OUP], BF16)
        ident_sb = const.tile([128, 32], BF16)
        expb_sb = const.tile([128, NJC, N], BF16)
        for kc in range(3):
            nc.sync.dma_start(wqkv_sb[:, kc, :], wqkv_d.ap()[kc])
        nc.sync.dma_start(wout_sb[:], wout4_d.ap())
        nc.sync.dma_start(ident_sb[:], ident_d.ap())
        nc.sync.dma_start(expb_sb[:], expb_d.ap())

        # Resident activation layouts
        QKV = big.tile([96, TOK], BF16)              # rows: qT 0:32, kT 32:64, vT 64:96
        K0 = big.tile([32, TOK], BF16)               # kT re-homed to partitions 0:32
        V_sb = big.tile([128, NB * NJC * 33], BF16)  # v natural [j,d] per (b,jc) + ones col
        OT = big.tile([33, TOK], BF16)               # attn out^T (+ denom row 32)
        den_nat = big.tile([128, NB * NJC], BF16)    # denominators, natural layout
        recip_nat = big.tile([128, NB * NJC], F32)

        nc.gpsimd.memset(V_sb[:], 1.0)  # ones column pre-fill; v blocks overwritten

        xt_pool = ctx.enter_context(tc.tile_pool(name="xt", bufs=8))
        es_pool = ctx.enter_context(tc.tile_pool(name="es", bufs=3))
        pt_pool = ctx.enter_context(tc.tile_pool(name="pt", bufs=2 * NJG + 2))
        out_pool = ctx.enter_context(tc.tile_pool(name="outp", bufs=6))

        # ---------------- Stage A: qkv projections + v transposes ----------------
        loopA = tc.For_i(0, loop_k, 1) if loop_k else nullcontext()
        with tc.tile_pool(name="ps_qkv", bufs=3, space="PSUM") as ps_qkv, \
             tc.tile_pool(name="ps_vt", bufs=1, space="PSUM") as ps_vt, loopA:
            xt_engs = [nc.sync, nc.scalar, nc.gpsimd, nc.scalar]
            for tch in range(TOK // IH):
                xt_t = xt_pool.tile([128, 3, IH], BF16, tag="xt")
                # spread loads across engine DMA queues for bus parallelism
                xt_engs[tch % 4].dma_start(
                    xt_t[:],
                    xt_d.ap()[:, tch * IH:(tch + 1) * IH].rearrange(
                        "(c p) q -> p c q", p=128))
                ps = ps_qkv.tile([128, IH], F32, tag="ps_qkv")
                for kc in range(3):
                    nc.tensor.matmul(ps[0:96, :], wqkv_sb[:, kc, :],
                                     xt_t[:, kc, :],
                                     start=(kc == 0), stop=(kc == 2))
                nc.vector.tensor_copy(QKV[:, tch * IH:(tch + 1) * IH], ps[0:96, :])
            # re-home kT to partitions 0:32 (DMA crosses partitions)
            nc.sync.dma_start(K0[:], QKV[32:64, :])
            # v transposes: vT [32,128] blocks -> v natural [128,32] per (b,jc)
            TG = min(4, NJC)
            for b in range(NB):
                for jg in range(NJC // TG):
                    vt = ps_vt.tile([128, 4, 1024], BF16, tag="ps_vt")
                    for r in range(TG):
                        jc = jg * TG + r
                        nc.tensor.transpose(
                            vt[:, r, 0:32],
                            QKV[64:96, b * N + jc * 128:b * N + (jc + 1) * 128],
                            ident_sb[64:96, 0:32],
                            tile_position=(64, 0))
                    vv = V_sb[:].rearrange("p (b j e) -> p b j e", j=NJC, e=33)
                    nc.vector.tensor_copy(
                        vv[:, b, jg * TG:jg * TG + TG, 0:32],
                        vt[:, 0:TG, 0:32])

        # ---------------- Stage B: attention + output projection ----------------
        loopB = tc.For_i(0, loop_k, 1) if loop_k else nullcontext()
        with tc.tile_pool(name="ps_dots", bufs=2, space="PSUM") as ps_dots, \
             tc.tile_pool(name="ps_av", bufs=2, space="PSUM") as ps_av, \
             tc.tile_pool(name="ps_out", bufs=2, space="PSUM") as ps_out, loopB:
            vv = V_sb[:].rearrange("p (b j e) -> p b j e", j=NJC, e=33)

            def tail1(b, ih, pts):
                """AV + evac + denominator spread/reciprocal for one unit."""
                i0 = b * N + ih * IH
                av = ps_av.tile([128, IH], F32, tag="ps_av")
                for jc in range(NJC):
                    nc.tensor.matmul(
                        av[0:33, :], vv[:, b, jc, 0:33],
                        pts[jc // JG][:, (jc % JG) * IH:(jc % JG + 1) * IH],
                        start=(jc == 0), stop=(jc == NJC - 1))
                nc.vector.tensor_copy(OT[:, i0:i0 + IH], av[0:33, :])
                # denominator row -> natural layout via tiny PE transposes
                # (a DMA spread would serialize on the HWDGE queue)
                dent = ps_av.tile([128, IH], F32, tag="ps_av")
                dv = dent[:, 0:4].bitcast(BF16)          # [128, 8] bf16
                for tcl in range(NTC):
                    nc.tensor.transpose(
                        dv[:, 2 * tcl:2 * tcl + 1],
                        OT[32:33, i0 + tcl * 128:i0 + (tcl + 1) * 128],
                        ident_sb[32:33, 0:1],
                        tile_position=(32, 0))
                nc.vector.tensor_copy(
                    den_nat[:, b * NJC + ih * NTC:b * NJC + (ih + 1) * NTC],
                    dv[:, 0:8:2])
                nc.vector.reciprocal(
                    recip_nat[:, b * NJC + ih * NTC:b * NJC + (ih + 1) * NTC],
                    den_nat[:, b * NJC + ih * NTC:b * NJC + (ih + 1) * NTC])

            def tail2(b, ih, pts):
                """Output projection + normalize for one unit."""
                i0 = b * N + ih * IH
                ot = None
                for tcl in range(NTC):
                    po = ps_out.tile([128, 512], F32, tag="ps_out")
                    nc.tensor.matmul(
                        po[:, 0:OUP],
                        OT[0:32, i0 + tcl * 128:i0 + (tcl + 1) * 128],
                        wout_sb[0:32, :],
                        start=True, stop=True)
                    if tcl % 2 == 0:
                        ot = out_pool.tile([128, 2, OUP], BF16, tag="outp")
                    rc = recip_nat[:, b * NJC + ih * NTC + tcl:
                                   b * NJC + ih * NTC + tcl + 1]
                    if (ih * NTC + tcl) % 5 == 4:
                        nc.scalar.activation(ot[:, tcl % 2, :], po[:, 0:OUP],
                                             AF.Copy, scale=rc)
                    else:
                        nc.vector.tensor_scalar_mul(ot[:, tcl % 2, :],
                                                    po[:, 0:OUP], rc)
                    if tcl % 2 == 1:
                        oeng = [nc.sync, nc.gpsimd][
                            (b * NJC + ih * NTC + tcl) % 2]
                        oeng.dma_start(
                            outp_d.ap()[i0 + (tcl - 1) * 128:
                                        i0 + (tcl + 1) * 128, :].rearrange(
                                "(d p) f -> p d f", p=128),
                            ot[:])

            q = []  # 2-deep software pipeline: tail1 at U-1, tail2 at U-2
            mi = 0
            for b in range(NB):
                for ih in range(NIH):
                    i0 = b * N + ih * IH      # token offset of this query tile
                    pts = []
                    for jg in range(NJG):
                        ps = ps_dots.tile([128, JG * IH], F32, tag="ps_dots")
                        for r in range(JG):
                            jc = jg * JG + r
                            nc.tensor.matmul(
                                ps[:, r * IH:(r + 1) * IH],
                                K0[:, b * N + jc * 128:b * N + (jc + 1) * 128],
                                QKV[0:32, i0:i0 + IH],
                                start=True, stop=True)
                        es = es_pool.tile([128, JG * IH], BF16, tag="es")
                        nc.scalar.activation(es[:], ps[:], AF.Exp, scale=float(SCALE))
                        pt = pt_pool.tile([128, JG * IH], BF16, tag="pt")
                        for r in range(JG):
                            jc = jg * JG + r
                            eng = nc.vector if (mi % 2 == 0) else nc.gpsimd
                            mi += 1
                            eng.tensor_mul(
                                pt[:, r * IH:(r + 1) * IH],
                                es[:, r * IH:(r + 1) * IH],
                                expb_sb[:, jc, ih * IH:(ih + 1) * IH])
                        pts.append(pt)
                    q.append((b, ih, pts))
                    if len(q) >= 2:
                        tail1(*q[-2])
                        tail2(*q[-2])
            tail1(*q[-1])
            tail2(*q[-1])
    nc.compile()
    return nc


def host_prep(x, w_qkv, relative_bias_table, relative_index, w_out, NB, N):
    """Build per-core input maps."""
    bf = ml_dtypes.bfloat16
    TOK = NB * N
    NJC = N // 128
    xt = np.ascontiguousarray(x.reshape(TOK, INP).T).astype(bf)
    ident = np.tile(np.eye(32, dtype=np.float32), (4, 1)).astype(bf)
    bias_full = relative_bias_table[relative_index]  # [N, N, H]
    in_maps = []
    for h in range(HEADS):
        w96 = np.concatenate(
            [w_qkv[:, h * D:(h + 1) * D],
             w_qkv[:, 256 + h * D:256 + (h + 1) * D],
             w_qkv[:, 512 + h * D:512 + (h + 1) * D]], axis=1)  # [384, 96]
        wqkv3 = np.ascontiguousarray(w96.reshape(3, 128, 96)).astype(bf)
        wout4 = np.tile(w_out[h * D:(h + 1) * D, :], (4, 1)).astype(bf)
        expbT = np.exp(bias_full[:, :, h].T)  # [j, i]
        expb = np.ascontiguousarray(
            expbT.reshape(NJC, 128, N).transpose(1, 0, 2)).astype(bf)
        in_maps.append({
            "xt": xt, "wqkv": wqkv3, "wout4": wout4,
            "expb": expb, "ident": ident,
        })
    return in_maps


_NC_CACHE = {}


def kernel(x, w_qkv, relative_bias_table, w_out, b_out, relative_index):
    x = np.asarray(x, dtype=np.float32)
    w_qkv = np.asarray(w_qkv, dtype=np.float32)
    relative_bias_table = np.asarray(relative_bias_table, dtype=np.float32)
    w_out = np.asarray(w_out, dtype=np.float32)
    b_out = np.asarray(b_out, dtype=np.float32)
    relative_index = np.asarray(relative_index)

    NB, N, _ = x.shape
    key = (NB, N)
    if key not in _NC_CACHE:
        _NC_CACHE[key] = build_kernel(NB=NB, N=N, num_devices=HEADS)
    nc = _NC_CACHE[key]

    in_maps = host_prep(x, w_qkv, relative_bias_table, relative_index, w_out, NB, N)
    res = run_bass_kernel_spmd(nc, in_maps, core_ids=list(range(HEADS)))
    out = np.zeros((NB * N, OUP), np.float32)
    for r in res.results:
        out += r["outp"].astype(np.float32)
    out += b_out[None, :]
    return out.reshape(NB, N, OUP)

